# revision 51
# baseline (speedup 1.0000x reference)
"""DiversifiedSelfAttention Trainium2 kernel.

Sharding: 8 cores = 2 batches x 4 head-groups (4 heads each).
Each core computes q/k/v projections for its 4 heads, causal softmax attention
(both the full [B,H,L,L] attention-probability output and the attention-weighted
values), and a partial output projection. Host sums partials per batch.

Self-contained: hardcodes B=2, L=2048, D=1024, H=16, HD=64.
"""
import numpy as np

import concourse.bass as bass
import concourse.mybir as mybir
from concourse import bacc
from concourse.tile import TileContext
from concourse.bass_utils import run_bass_kernel_spmd

F32 = mybir.dt.float32
F32R = mybir.dt.float32r
AX = mybir.AluOpType
AF = mybir.ActivationFunctionType

B, L, D, H = 2, 2048, 1024, 16
HD = D // H          # 64
NCORE = 8
HPC = H // 4         # heads per core = 4
DG = HPC * HD        # per-core projection width = 256
NT = L // 128        # 16 q/k tiles of 128
NB = L // 512        # 4 blocks of 512
EXP_NEG50 = float(np.exp(np.float32(-50.0)))

_cache = {}


def _chunks(nblocks):
    """Split nblocks 512-blocks into chunks of <=2 blocks: [(block_off, nblk)]."""
    out = []
    off = 0
    while nblocks > 0:
        w = min(2, nblocks)
        out.append((off, w))
        off += w
        nblocks -= w
    return out


def build(l=L):
    NT = l // 128
    NB = l // 512
    L = l
    nc = bacc.Bacc()

    xT = nc.dram_tensor("xT", [D, L], F32, kind="ExternalInput")
    wq = nc.dram_tensor("wq", [D, DG], F32, kind="ExternalInput")
    wk = nc.dram_tensor("wk", [D, DG], F32, kind="ExternalInput")
    wv = nc.dram_tensor("wv", [D, DG], F32, kind="ExternalInput")
    wo = nc.dram_tensor("wo", [DG, D], F32, kind="ExternalInput")
    bq = nc.dram_tensor("bq", [DG, 1], F32, kind="ExternalInput")
    bk = nc.dram_tensor("bk", [DG, 1], F32, kind="ExternalInput")
    bv = nc.dram_tensor("bv", [DG, 1], F32, kind="ExternalInput")
    # additive masks for the 4 diagonal alignments (rel = tile*128 - block*512)
    mq = nc.dram_tensor("mq", [128, 4, 512], F32, kind="ExternalInput")  # [q,k] min-mask: -50 where k>q else +big
    mk = nc.dram_tensor("mk", [128, 4, 512], F32, kind="ExternalInput")  # [k,q]: -300 where k>q
    attn = nc.dram_tensor("attn", [HPC, L, L], F32, kind="ExternalOutput")
    outT = nc.dram_tensor("outT", [D, L], F32, kind="ExternalOutput")

    with TileContext(nc) as tc:
        import contextlib
        ctx = contextlib.ExitStack()
        with ctx:
            pool = lambda name, bufs, space="SBUF": ctx.enter_context(
                tc.tile_pool(name=name, bufs=bufs, space=space))

            # persistent pools (whole kernel)
            p_qt = pool("qt", 2)                # QT f32r 2x[128, L]
            p_kt = pool("kt", 2)                # KT f32r 2x[128, L]
            p_v = pool("v", 16)                 # V f32r 16x[128, DG]
            p_otn = pool("otn", 4)              # normalized O^T f32r 4x[64, L]
            p_sm = pool("sm", 16)                # small: rr/rcp/accs
            p_one = pool("one", 1)              # small constants/biases
            p_wor = pool("wor", 4)              # rounded wo f32r 4x[64, D]

            ctxA = contextlib.ExitStack()       # QKV-phase transient pools
            poolA = lambda name, bufs, space="SBUF": ctxA.enter_context(
                tc.tile_pool(name=name, bufs=bufs, space=space))
            p_stage = poolA("stage", 2)         # [128, L] f32 dma staging
            p_xr = poolA("xr", 8)               # xT f32r, 8x[128, L]
            p_wst = poolA("wst", 2)             # weight f32 staging [128, 256]
            p_wr = poolA("wr", 24)              # rounded qkv weights f32r
            ps_pr = poolA("ps_pr", 2, "PSUM")   # projections [128, 512]

            # ---- load constants ----
            bias_sb = {}
            for nm, t in (("bq", bq), ("bk", bk), ("bv", bv)):
                b_t = p_one.tile([128, 2], F32, tag=f"b{nm}")
                for i in range(2):
                    nc.sync.dma_start(out=b_t[:, i:i + 1], in_=t[i * 128:(i + 1) * 128, :])
                bias_sb[nm] = b_t
            # pre-scaled bq for the ACT Identity epilogue: Exp(s) uses q*0.125
            bqs = p_one.tile([128, 2], F32, tag="bqs")
            nc.vector.tensor_scalar(out=bqs, in0=bias_sb["bq"], scalar1=0.125,
                                    scalar2=None, op0=AX.mult)
            bias_sb["bq"] = bqs

            # ---- load + round wo (needed only at the end, loaded early) ----
            wo_r = []
            for kc in range(HPC):
                wst = p_stage.tile([128, L], F32, tag="stage")
                nc.sync.dma_start(out=wst[0:64, 0:D], in_=wo[kc * 64:(kc + 1) * 64, :])
                rt = p_wor.tile([64, D], F32R, tag="wor", name=f"wor{kc}")
                nc.vector.tensor_copy(out=rt, in_=wst[0:64, 0:D])
                wo_r.append(rt)

            # ---- load + round xT ----
            xr = []
            for i in range(D // 128):
                st = p_stage.tile([128, L], F32, tag="stage")
                nc.sync.dma_start(out=st, in_=xT[i * 128:(i + 1) * 128, :])
                xt = p_xr.tile([128, L], F32R, tag="xr")
                nc.vector.tensor_copy(out=xt, in_=st)
                xr.append(xt)

            # ---- load + round projection weights ----
            wr = {}
            for nm, t in (("wq", wq), ("wk", wk), ("wv", wv)):
                tiles = []
                for i in range(D // 128):
                    stw = p_wst.tile([128, DG], F32, tag="wst")
                    nc.sync.dma_start(out=stw, in_=t[i * 128:(i + 1) * 128, :])
                    rt = p_wr.tile([128, DG], F32R, tag="wr")
                    nc.vector.tensor_copy(out=rt, in_=stw)
                    tiles.append(rt)
                wr[nm] = tiles

            # ---- QT / KT / V projections ----
            # Order: QT/KT tile 0 (heads 0-1) first so attention can start,
            # then V, then QT/KT tile 1.
            qt = [None, None]
            kt = [None, None]

            def project_qk(nm, dst, scale, mt):
                big = (p_qt if nm == "wq" else p_kt).tile(
                    [128, L], F32R, tag=("qt" if nm == "wq" else "kt"),
                    name=f"{nm}_{mt}")
                for nb_i in range(L // 512):
                    ps = ps_pr.tile([128, 512], F32, tag="ps_pr", name="ps_prt")
                    for kc in range(D // 128):
                        nc.tensor.matmul(
                            out=ps,
                            lhsT=wr[nm][kc][:, mt * 128:(mt + 1) * 128],
                            rhs=xr[kc][:, nb_i * 512:(nb_i + 1) * 512],
                            start=(kc == 0), stop=(kc == D // 128 - 1))
                    bn = "bq" if nm == "wq" else "bk"
                    nc.vector.tensor_scalar(
                        out=big[:, nb_i * 512:(nb_i + 1) * 512], in0=ps,
                        scalar1=bias_sb[bn][:, mt:mt + 1], scalar2=scale,
                        op0=AX.add, op1=AX.mult)
                dst[mt] = big

            project_qk("wq", qt, 0.125, 0)
            project_qk("wk", kt, 1.0, 0)
            VLAST = False
            if not VLAST:
                project_qk("wq", qt, 0.125, 1)
                project_qk("wk", kt, 1.0, 1)

            vt = []
            bvb = p_one.tile([128, DG], F32, tag="bvb")
            nc.gpsimd.dma_start(
                out=bvb,
                in_=bv[:, :].rearrange("a b -> b a").broadcast_to((128, DG)))
            for lt in range(NT):
                ps = ps_pr.tile([128, 512], F32, tag="ps_pr", name="ps_prt")
                for kc in range(D // 128):
                    nc.tensor.matmul(
                        out=ps[:, 0:DG],
                        lhsT=xr[kc][:, lt * 128:(lt + 1) * 128],
                        rhs=wr["wv"][kc],
                        start=(kc == 0), stop=(kc == D // 128 - 1))
                v_t = p_v.tile([128, DG], F32R, tag="v")
                nc.vector.scalar_tensor_tensor(
                    out=v_t, in0=ps[:, 0:DG], scalar=1.0, in1=bvb,
                    op0=AX.mult, op1=AX.add)
                vt.append(v_t)
            if VLAST:
                project_qk("wq", qt, 0.125, 1)
                project_qk("wk", kt, 1.0, 1)

            ctxA.close()

            # ---- attention-phase pools ----
            ctxB = contextlib.ExitStack()
            poolB = lambda name, bufs, space="SBUF": ctxB.enter_context(
                tc.tile_pool(name=name, bufs=bufs, space=space))
            p_msk = poolB("msk", 1)             # masks
            p_att = poolB("att", 6)             # [q,k] exp/attn staging f32 [128, 2048]
            mq_sb = p_msk.tile([128, 4, 512], F32, tag="mq")
            mk_sb = p_msk.tile([128, 4, 512], F32, tag="mk")
            nc.sync.dma_start(out=mq_sb, in_=mq[:, :, :])
            nc.sync.dma_start(out=mk_sb, in_=mk[:, :, :])
            cexp = p_msk.tile([128, 512], F32, tag="cexp")
            nc.vector.memset(cexp, EXP_NEG50)
            ones_f = p_msk.tile([1, 64], F32, tag="ones_f")
            nc.vector.memset(ones_f, 1.0)
            p_ext = poolB("ext", 6)             # [k,q] exp f32r [128, 512]
            p_fill = poolB("fill", 4)           # fill tiles [128, 512]
            p_rb = poolB("rb", 4)               # r row + r broadcast [64, 512]
            ps_qk = poolB("ps_qk", 2, "PSUM")   # [128, 1024] = 2 banks each
            ps_kq = poolB("ps_kq", 2, "PSUM")   # [128, 512]
            ps_ot = poolB("ps_ot", 2, "PSUM")   # [64, 512]

            # ---- per-head attention ----
            otn = [p_otn.tile([64, L], F32R, tag="otn", name=f"otn{i}")
                   for i in range(HPC)]
            for h in range(HPC):
                hb = (h % 2) * 64          # partition base within qt/kt tiles
                ht = h // 2
                qth = qt[ht]
                kth = kt[ht]
                rcp = p_sm.tile([128, NT], F32, tag="rcp")

                # --- [q,k] orientation: attn output + row sums ---
                # No clamp off the diagonal: |scores| << 50 so clip(s,+-50)=s;
                # on the diagonal block a single fused (add mask, max -50)
                # reproduces the reference's clip(s + -inf_mask, -50, 50).
                for t in range(NT):
                    dj = t // 4                     # diagonal 512-block index
                    ext = (dj + 1) * 512
                    at = p_att.tile([128, 2048], F32, tag="att")
                    accs = []
                    for ci, (boff, nblk) in enumerate(_chunks(dj + 1)):
                        w = nblk * 512
                        ps = ps_qk.tile([128, 1024], F32, tag="ps_qk")
                        for bi in range(nblk):
                            kb = boff + bi
                            nc.tensor.matmul(
                                out=ps[:, bi * 512:(bi + 1) * 512],
                                lhsT=qth[hb:hb + 64, t * 128:(t + 1) * 128],
                                rhs=kth[hb:hb + 64, kb * 512:(kb + 1) * 512],
                                start=True, stop=True)
                        if boff + nblk - 1 == dj:
                            dlo = (nblk - 1) * 512
                            nc.vector.tensor_tensor(
                                out=ps[:, dlo:w], in0=ps[:, dlo:w],
                                in1=mq_sb[:, t % 4, :], op=AX.min)
                        acc = p_sm.tile([128, 2], F32, tag="acc")
                        nc.scalar.activation(
                            out=at[:, boff * 512:boff * 512 + w], in_=ps[:, 0:w],
                            func=AF.Exp, accum_out=acc[:, 0:1])
                        accs.append(acc)
                    # row sum -> rcp[:, t]
                    if len(accs) == 1:
                        nc.vector.reciprocal(out=rcp[:, t:t + 1], in_=accs[0][:, 0:1])
                    else:
                        ssum = p_sm.tile([128, 2], F32, tag="acc")
                        nc.vector.tensor_tensor(
                            out=ssum[:, 0:1], in0=accs[0][:, 0:1],
                            in1=accs[1][:, 0:1], op=AX.add)
                        nc.vector.reciprocal(out=rcp[:, t:t + 1], in_=ssum[:, 0:1])
                    # normalize + single store of the computed span
                    nc.vector.tensor_scalar(
                        out=at[:, 0:ext], in0=at[:, 0:ext],
                        scalar1=rcp[:, t:t + 1], scalar2=None, op0=AX.mult)
                    st_eng = nc.sync if t % 2 == 0 else nc.gpsimd
                    st_eng.dma_start(
                        out=attn[h, t * 128:(t + 1) * 128, 0:ext],
                        in_=at[:, 0:ext])
                    # masked fill: one broadcast DMA over the masked span
                    nmask = NB - (dj + 1)
                    if nmask > 0:
                        ft = p_fill.tile([128, 512], F32, tag="fill")
                        nc.gpsimd.tensor_scalar(
                            out=ft, in0=cexp, scalar1=rcp[:, t:t + 1],
                            scalar2=None, op0=AX.mult)
                        fap = ft[:, :]
                        src = bass.AP(
                            tensor=fap.tensor, offset=fap.offset,
                            ap=[fap.ap[0], [0, nmask], fap.ap[1]])
                        nc.gpsimd.dma_start(
                            out=attn[h, t * 128:(t + 1) * 128, ext:L],
                            in_=src)

                # --- [k,q] orientation: O^T = V^T exp(S^T), normalized on copy ---
                for j in range(NB):
                    ot_ps = ps_ot.tile([64, 512], F32, tag="ps_ot")
                    # r for this q block: SBUF->SBUF transposing DMA packs
                    # rcp[:, 4j:4j+4] (partition-major) into a [1, 512] row,
                    # then PE outer-product with ones broadcasts it across
                    # 64 partitions.
                    rrow = p_rb.tile([1, 512], F32, tag="rrow")
                    for tt in range(4):
                        nc.sync.dma_start(
                            out=rrow[0:1, tt * 128:(tt + 1) * 128],
                            in_=rcp[:, 4 * j + tt:4 * j + tt + 1])
                    rps = ps_kq.tile([128, 512], F32, tag="ps_kq")
                    nc.tensor.matmul(out=rps[0:64, :], lhsT=ones_f,
                                     rhs=rrow, start=True, stop=True)
                    rb = p_rb.tile([64, 512], F32, tag="rb")
                    nc.vector.tensor_copy(out=rb, in_=rps[0:64, :])
                    ntk = 4 * j + 4        # computed k tiles for this q block
                    for tk in range(ntk):
                        ps2 = ps_kq.tile([128, 512], F32, tag="ps_kq")
                        nc.tensor.matmul(
                            out=ps2,
                            lhsT=kth[hb:hb + 64, tk * 128:(tk + 1) * 128],
                            rhs=qth[hb:hb + 64, j * 512:(j + 1) * 512],
                            start=True, stop=True)
                        if tk // 4 == j:   # diagonal block: mask k > q
                            nc.vector.tensor_tensor(
                                out=ps2, in0=ps2, in1=mk_sb[:, tk % 4, :],
                                op=AX.add)
                        ex = p_ext.tile([128, 512], F32R, tag="ext")
                        nc.scalar.activation(out=ex, in_=ps2, func=AF.Exp)
                        nc.tensor.matmul(
                            out=ot_ps,
                            lhsT=vt[tk][:, h * 64:(h + 1) * 64],
                            rhs=ex, start=(tk == 0), stop=(tk == ntk - 1))
                    # copy out normalized O^T for this q block
                    nc.vector.tensor_tensor(
                        out=otn[h][:, j * 512:(j + 1) * 512],
                        in0=ot_ps,
                        in1=rb,
                        op=AX.mult)

            # ---- output projection: outT[d, q] = sum_m wo[m, d] otn[m, q] ----
            # Chunk-major so early q blocks flow as soon as the last head's
            # otn slices land; reuses attention-phase psum/staging pools to
            # avoid an address-reuse barrier at the phase boundary.
            for (boff, nblk) in _chunks(NB):
                w = nblk * 512
                for mt in range(D // 128):
                    ps = ps_qk.tile([128, 1024], F32, tag="ps_qk", name="ps_op")
                    for bi in range(nblk):
                        qb = boff + bi
                        for kc in range(HPC):
                            nc.tensor.matmul(
                                out=ps[:, bi * 512:(bi + 1) * 512],
                                lhsT=wo_r[kc][:, mt * 128:(mt + 1) * 128],
                                rhs=otn[kc][:, qb * 512:(qb + 1) * 512],
                                start=(kc == 0), stop=(kc == HPC - 1))
                    ost = p_att.tile([128, 2048], F32, tag="att", name="ostt")
                    nc.vector.tensor_copy(out=ost[:, 0:w], in_=ps[:, 0:w])
                    nc.scalar.dma_start(
                        out=outT[mt * 128:(mt + 1) * 128,
                                 boff * 512:boff * 512 + w],
                        in_=ost[:, 0:w])
            ctxB.close()

    nc.finalize()
    return nc


def _masks():
    p = np.arange(128)
    jj = np.arange(512)
    mq = np.zeros((128, 4, 512), np.float32)
    mk = np.zeros((128, 4, 512), np.float32)
    for r in range(4):
        rel = r * 128
        mq[:, r, :] = np.where((rel + p)[:, None] < jj[None, :], -50.0, 3.0e38)
        mk[:, r, :] = np.where((rel + p)[:, None] > jj[None, :], -300.0, 0.0)
    return mq, mk


_last_result = None


def kernel(x, wq, bq, wk, bk, wv, bv, wo, bo):
    global _last_result
    if "nc" not in _cache:
        _cache["nc"] = build()
        _cache["masks"] = _masks()
    nc = _cache["nc"]
    mq, mk = _cache["masks"]

    x = np.asarray(x, np.float32)
    in_maps = []
    for c in range(NCORE):
        b, g = divmod(c, 4)
        sl = slice(g * DG, (g + 1) * DG)
        in_maps.append({
            "xT": np.ascontiguousarray(x[b].T),
            "wq": np.ascontiguousarray(np.asarray(wq, np.float32)[:, sl]),
            "wk": np.ascontiguousarray(np.asarray(wk, np.float32)[:, sl]),
            "wv": np.ascontiguousarray(np.asarray(wv, np.float32)[:, sl]),
            "wo": np.ascontiguousarray(np.asarray(wo, np.float32)[sl, :]),
            "bq": np.ascontiguousarray(np.asarray(bq, np.float32)[sl, None]),
            "bk": np.ascontiguousarray(np.asarray(bk, np.float32)[sl, None]),
            "bv": np.ascontiguousarray(np.asarray(bv, np.float32)[sl, None]),
            "mq": mq, "mk": mk,
        })
    res = run_bass_kernel_spmd(nc, in_maps, core_ids=list(range(NCORE)))
    _last_result = res

    out = np.zeros((B, L, D), np.float32)
    attn = np.empty((B, H, L, L), np.float32)
    for c in range(NCORE):
        b, g = divmod(c, 4)
        r = res.results[c]
        attn[b, g * HPC:(g + 1) * HPC] = r["attn"]
        out[b] += r["outT"].T
    out += np.asarray(bo, np.float32)[None, None, :]
    return out, attn


# revision 59
# speedup vs baseline: 1.0053x; 1.0053x over previous
"""DiversifiedSelfAttention Trainium2 kernel.

Sharding: 8 cores = 2 batches x 4 head-groups (4 heads each).
Each core computes q/k/v projections for its 4 heads, causal softmax attention
(both the full [B,H,L,L] attention-probability output and the attention-weighted
values), and a partial output projection. Host sums partials per batch.

Self-contained: hardcodes B=2, L=2048, D=1024, H=16, HD=64.
"""
import numpy as np

import concourse.bass as bass
import concourse.mybir as mybir
from concourse import bacc
from concourse.tile import TileContext
from concourse.bass_utils import run_bass_kernel_spmd

F32 = mybir.dt.float32
F32R = mybir.dt.float32r
AX = mybir.AluOpType
AF = mybir.ActivationFunctionType

B, L, D, H = 2, 2048, 1024, 16
HD = D // H          # 64
NCORE = 8
HPC = H // 4         # heads per core = 4
DG = HPC * HD        # per-core projection width = 256
NT = L // 128        # 16 q/k tiles of 128
NB = L // 512        # 4 blocks of 512
EXP_NEG50 = float(np.exp(np.float32(-50.0)))

_cache = {}


def _chunks(nblocks):
    """Split nblocks 512-blocks into chunks of <=2 blocks: [(block_off, nblk)]."""
    out = []
    off = 0
    while nblocks > 0:
        w = min(2, nblocks)
        out.append((off, w))
        off += w
        nblocks -= w
    return out


def build(l=L):
    NT = l // 128
    NB = l // 512
    L = l
    nc = bacc.Bacc()

    xT = nc.dram_tensor("xT", [D, L], F32, kind="ExternalInput")
    wq = nc.dram_tensor("wq", [D, DG], F32, kind="ExternalInput")
    wk = nc.dram_tensor("wk", [D, DG], F32, kind="ExternalInput")
    wv = nc.dram_tensor("wv", [D, DG], F32, kind="ExternalInput")
    wo = nc.dram_tensor("wo", [DG, D], F32, kind="ExternalInput")
    bq = nc.dram_tensor("bq", [DG, 1], F32, kind="ExternalInput")
    bk = nc.dram_tensor("bk", [DG, 1], F32, kind="ExternalInput")
    bv = nc.dram_tensor("bv", [DG, 1], F32, kind="ExternalInput")
    # additive masks for the 4 diagonal alignments (rel = tile*128 - block*512)
    mq = nc.dram_tensor("mq", [128, 4, 512], F32, kind="ExternalInput")  # [q,k] min-mask: -50 where k>q else +big
    mk = nc.dram_tensor("mk", [128, 4, 512], F32, kind="ExternalInput")  # [k,q]: -300 where k>q
    attn = nc.dram_tensor("attn", [HPC, L, L], F32, kind="ExternalOutput")
    outT = nc.dram_tensor("outT", [D, L], F32, kind="ExternalOutput")

    with TileContext(nc) as tc:
        import contextlib
        ctx = contextlib.ExitStack()
        with ctx:
            pool = lambda name, bufs, space="SBUF": ctx.enter_context(
                tc.tile_pool(name=name, bufs=bufs, space=space))

            # persistent pools (whole kernel)
            p_qt = pool("qt", 2)                # QT f32r 2x[128, L]
            p_kt = pool("kt", 2)                # KT f32r 2x[128, L]
            p_v = pool("v", 16)                 # V f32r 16x[128, DG]
            p_otn = pool("otn", 4)              # normalized O^T f32r 4x[64, L]
            p_sm = pool("sm", 16)                # small: rr/rcp/accs
            p_one = pool("one", 1)              # small constants/biases
            p_wor = pool("wor", 4)              # rounded wo f32r 4x[64, D]

            ctxA = contextlib.ExitStack()       # QKV-phase transient pools
            poolA = lambda name, bufs, space="SBUF": ctxA.enter_context(
                tc.tile_pool(name=name, bufs=bufs, space=space))
            p_stage = poolA("stage", 2)         # [128, L] f32 dma staging
            p_xr = poolA("xr", 8)               # xT f32r, 8x[128, L]
            p_wst = poolA("wst", 2)             # weight f32 staging [128, 256]
            p_wr = poolA("wr", 24)              # rounded qkv weights f32r
            ps_pr = poolA("ps_pr", 2, "PSUM")   # projections [128, 512]

            # ---- load constants ----
            bias_sb = {}
            for nm, t in (("bq", bq), ("bk", bk), ("bv", bv)):
                b_t = p_one.tile([128, 2], F32, tag=f"b{nm}")
                for i in range(2):
                    nc.sync.dma_start(out=b_t[:, i:i + 1], in_=t[i * 128:(i + 1) * 128, :])
                bias_sb[nm] = b_t
            # pre-scaled bq for the ACT Identity epilogue: Exp(s) uses q*0.125
            bqs = p_one.tile([128, 2], F32, tag="bqs")
            nc.vector.tensor_scalar(out=bqs, in0=bias_sb["bq"], scalar1=0.125,
                                    scalar2=None, op0=AX.mult)
            bias_sb["bq"] = bqs

            # ---- load + round wo (needed only at the end, loaded early) ----
            wo_r = []
            for kc in range(HPC):
                wst = p_stage.tile([128, L], F32, tag="stage")
                nc.sync.dma_start(out=wst[0:64, 0:D], in_=wo[kc * 64:(kc + 1) * 64, :])
                rt = p_wor.tile([64, D], F32R, tag="wor", name=f"wor{kc}")
                nc.vector.tensor_copy(out=rt, in_=wst[0:64, 0:D])
                wo_r.append(rt)

            # ---- load + round xT ----
            xr = []
            for i in range(D // 128):
                st = p_stage.tile([128, L], F32, tag="stage")
                nc.sync.dma_start(out=st, in_=xT[i * 128:(i + 1) * 128, :])
                xt = p_xr.tile([128, L], F32R, tag="xr")
                nc.vector.tensor_copy(out=xt, in_=st)
                xr.append(xt)

            # ---- load + round projection weights ----
            wr = {}
            for nm, t in (("wq", wq), ("wk", wk), ("wv", wv)):
                tiles = []
                for i in range(D // 128):
                    stw = p_wst.tile([128, DG], F32, tag="wst")
                    nc.sync.dma_start(out=stw, in_=t[i * 128:(i + 1) * 128, :])
                    rt = p_wr.tile([128, DG], F32R, tag="wr")
                    nc.vector.tensor_copy(out=rt, in_=stw)
                    tiles.append(rt)
                wr[nm] = tiles

            # ---- QT / KT / V projections ----
            # Order: QT/KT tile 0 (heads 0-1) first so attention can start,
            # then V, then QT/KT tile 1.
            qt = [None, None]
            kt = [None, None]

            def project_qk(nm, dst, scale, mt):
                big = (p_qt if nm == "wq" else p_kt).tile(
                    [128, L], F32R, tag=("qt" if nm == "wq" else "kt"),
                    name=f"{nm}_{mt}")
                for nb_i in range(L // 512):
                    ps = ps_pr.tile([128, 512], F32, tag="ps_pr", name="ps_prt")
                    for kc in range(D // 128):
                        nc.tensor.matmul(
                            out=ps,
                            lhsT=wr[nm][kc][:, mt * 128:(mt + 1) * 128],
                            rhs=xr[kc][:, nb_i * 512:(nb_i + 1) * 512],
                            start=(kc == 0), stop=(kc == D // 128 - 1))
                    bn = "bq" if nm == "wq" else "bk"
                    nc.vector.tensor_scalar(
                        out=big[:, nb_i * 512:(nb_i + 1) * 512], in0=ps,
                        scalar1=bias_sb[bn][:, mt:mt + 1], scalar2=scale,
                        op0=AX.add, op1=AX.mult)
                dst[mt] = big

            project_qk("wq", qt, 0.125, 0)
            project_qk("wk", kt, 1.0, 0)
            VLAST = False
            if not VLAST:
                project_qk("wq", qt, 0.125, 1)
                project_qk("wk", kt, 1.0, 1)

            vt = []
            bvb = p_one.tile([128, DG], F32, tag="bvb")
            nc.gpsimd.dma_start(
                out=bvb,
                in_=bv[:, :].rearrange("a b -> b a").broadcast_to((128, DG)))
            for lt in range(NT):
                ps = ps_pr.tile([128, 512], F32, tag="ps_pr", name="ps_prt")
                for kc in range(D // 128):
                    nc.tensor.matmul(
                        out=ps[:, 0:DG],
                        lhsT=xr[kc][:, lt * 128:(lt + 1) * 128],
                        rhs=wr["wv"][kc],
                        start=(kc == 0), stop=(kc == D // 128 - 1))
                v_t = p_v.tile([128, DG], F32R, tag="v")
                nc.vector.scalar_tensor_tensor(
                    out=v_t, in0=ps[:, 0:DG], scalar=1.0, in1=bvb,
                    op0=AX.mult, op1=AX.add)
                vt.append(v_t)
            if VLAST:
                project_qk("wq", qt, 0.125, 1)
                project_qk("wk", kt, 1.0, 1)

            ctxA.close()

            # ---- attention-phase pools ----
            ctxB = contextlib.ExitStack()
            poolB = lambda name, bufs, space="SBUF": ctxB.enter_context(
                tc.tile_pool(name=name, bufs=bufs, space=space))
            p_msk = poolB("msk", 1)             # masks
            p_att = poolB("att", 7)             # [q,k] exp/attn staging f32 [128, 2048]
            mq_sb = p_msk.tile([128, 4, 512], F32, tag="mq")
            mk_sb = p_msk.tile([128, 4, 512], F32, tag="mk")
            nc.sync.dma_start(out=mq_sb, in_=mq[:, :, :])
            nc.sync.dma_start(out=mk_sb, in_=mk[:, :, :])
            cexp = p_msk.tile([128, 512], F32, tag="cexp")
            nc.vector.memset(cexp, EXP_NEG50)
            ones_f = p_msk.tile([1, 64], F32, tag="ones_f")
            nc.vector.memset(ones_f, 1.0)
            p_ext = poolB("ext", 5)             # [k,q] exp f32r [128, 512]
            p_fill = poolB("fill", 4)           # fill tiles [128, 512]
            p_rb = poolB("rb", 4)               # r row + r broadcast [64, 512]
            ps_qk = poolB("ps_qk", 2, "PSUM")   # [128, 1024] = 2 banks each
            ps_kq = poolB("ps_kq", 2, "PSUM")   # [128, 512]
            ps_ot = poolB("ps_ot", 2, "PSUM")   # [64, 512]

            # ---- per-head attention ----
            otn = [p_otn.tile([64, L], F32R, tag="otn", name=f"otn{i}")
                   for i in range(HPC)]
            for h in range(HPC):
                hb = (h % 2) * 64          # partition base within qt/kt tiles
                ht = h // 2
                qth = qt[ht]
                kth = kt[ht]
                rcp = p_sm.tile([128, NT], F32, tag="rcp")

                # --- [q,k] orientation: attn output + row sums ---
                # No clamp off the diagonal: |scores| << 50 so clip(s,+-50)=s;
                # on the diagonal block a single fused (add mask, max -50)
                # reproduces the reference's clip(s + -inf_mask, -50, 50).
                for t in range(NT):
                    dj = t // 4                     # diagonal 512-block index
                    ext = (dj + 1) * 512
                    at = p_att.tile([128, 2048], F32, tag="att")
                    accs = []
                    for ci, (boff, nblk) in enumerate(_chunks(dj + 1)):
                        w = nblk * 512
                        ps = ps_qk.tile([128, 1024], F32, tag="ps_qk")
                        for bi in range(nblk):
                            kb = boff + bi
                            nc.tensor.matmul(
                                out=ps[:, bi * 512:(bi + 1) * 512],
                                lhsT=qth[hb:hb + 64, t * 128:(t + 1) * 128],
                                rhs=kth[hb:hb + 64, kb * 512:(kb + 1) * 512],
                                start=True, stop=True)
                        if boff + nblk - 1 == dj:
                            dlo = (nblk - 1) * 512
                            nc.vector.tensor_tensor(
                                out=ps[:, dlo:w], in0=ps[:, dlo:w],
                                in1=mq_sb[:, t % 4, :], op=AX.min)
                        acc = p_sm.tile([128, 2], F32, tag="acc")
                        nc.scalar.activation(
                            out=at[:, boff * 512:boff * 512 + w], in_=ps[:, 0:w],
                            func=AF.Exp, accum_out=acc[:, 0:1])
                        accs.append(acc)
                    # row sum -> rcp[:, t]
                    if len(accs) == 1:
                        nc.vector.reciprocal(out=rcp[:, t:t + 1], in_=accs[0][:, 0:1])
                    else:
                        ssum = p_sm.tile([128, 2], F32, tag="acc")
                        nc.vector.tensor_tensor(
                            out=ssum[:, 0:1], in0=accs[0][:, 0:1],
                            in1=accs[1][:, 0:1], op=AX.add)
                        nc.vector.reciprocal(out=rcp[:, t:t + 1], in_=ssum[:, 0:1])
                    # normalize + single store of the computed span
                    nc.vector.tensor_scalar(
                        out=at[:, 0:ext], in0=at[:, 0:ext],
                        scalar1=rcp[:, t:t + 1], scalar2=None, op0=AX.mult)
                    st_eng = nc.sync if t % 2 == 0 else nc.gpsimd
                    st_eng.dma_start(
                        out=attn[h, t * 128:(t + 1) * 128, 0:ext],
                        in_=at[:, 0:ext])
                    # masked fill: one broadcast DMA over the masked span
                    nmask = NB - (dj + 1)
                    if nmask > 0:
                        ft = p_fill.tile([128, 512], F32, tag="fill")
                        nc.gpsimd.tensor_scalar(
                            out=ft, in0=cexp, scalar1=rcp[:, t:t + 1],
                            scalar2=None, op0=AX.mult)
                        fap = ft[:, :]
                        src = bass.AP(
                            tensor=fap.tensor, offset=fap.offset,
                            ap=[fap.ap[0], [0, nmask], fap.ap[1]])
                        nc.gpsimd.dma_start(
                            out=attn[h, t * 128:(t + 1) * 128, ext:L],
                            in_=src)

                # --- [k,q] orientation: O^T = V^T exp(S^T), normalized on copy ---
                for j in range(NB):
                    ot_ps = ps_ot.tile([64, 512], F32, tag="ps_ot")
                    # r for this q block: SBUF->SBUF transposing DMA packs
                    # rcp[:, 4j:4j+4] (partition-major) into a [1, 512] row,
                    # then PE outer-product with ones broadcasts it across
                    # 64 partitions.
                    rrow = p_rb.tile([1, 512], F32, tag="rrow")
                    for tt in range(4):
                        nc.sync.dma_start(
                            out=rrow[0:1, tt * 128:(tt + 1) * 128],
                            in_=rcp[:, 4 * j + tt:4 * j + tt + 1])
                    rps = ps_kq.tile([128, 512], F32, tag="ps_kq")
                    nc.tensor.matmul(out=rps[0:64, :], lhsT=ones_f,
                                     rhs=rrow, start=True, stop=True)
                    rb = p_rb.tile([64, 512], F32, tag="rb")
                    nc.vector.tensor_copy(out=rb, in_=rps[0:64, :])
                    ntk = 4 * j + 4        # computed k tiles for this q block
                    for tk in range(ntk):
                        ps2 = ps_kq.tile([128, 512], F32, tag="ps_kq")
                        nc.tensor.matmul(
                            out=ps2,
                            lhsT=kth[hb:hb + 64, tk * 128:(tk + 1) * 128],
                            rhs=qth[hb:hb + 64, j * 512:(j + 1) * 512],
                            start=True, stop=True)
                        if tk // 4 == j:   # diagonal block: mask k > q
                            nc.vector.tensor_tensor(
                                out=ps2, in0=ps2, in1=mk_sb[:, tk % 4, :],
                                op=AX.add)
                        ex = p_ext.tile([128, 512], F32R, tag="ext")
                        nc.scalar.activation(out=ex, in_=ps2, func=AF.Exp)
                        nc.tensor.matmul(
                            out=ot_ps,
                            lhsT=vt[tk][:, h * 64:(h + 1) * 64],
                            rhs=ex, start=(tk == 0), stop=(tk == ntk - 1))
                    # copy out normalized O^T for this q block
                    nc.vector.tensor_tensor(
                        out=otn[h][:, j * 512:(j + 1) * 512],
                        in0=ot_ps,
                        in1=rb,
                        op=AX.mult)

            # ---- output projection: outT[d, q] = sum_m wo[m, d] otn[m, q] ----
            # Chunk-major so early q blocks flow as soon as the last head's
            # otn slices land; reuses attention-phase psum/staging pools to
            # avoid an address-reuse barrier at the phase boundary.
            for (boff, nblk) in _chunks(NB):
                w = nblk * 512
                for mt in range(D // 128):
                    ps = ps_qk.tile([128, 1024], F32, tag="ps_qk", name="ps_op")
                    for bi in range(nblk):
                        qb = boff + bi
                        for kc in range(HPC):
                            nc.tensor.matmul(
                                out=ps[:, bi * 512:(bi + 1) * 512],
                                lhsT=wo_r[kc][:, mt * 128:(mt + 1) * 128],
                                rhs=otn[kc][:, qb * 512:(qb + 1) * 512],
                                start=(kc == 0), stop=(kc == HPC - 1))
                    ost = p_att.tile([128, 2048], F32, tag="att", name="ostt")
                    nc.vector.tensor_copy(out=ost[:, 0:w], in_=ps[:, 0:w])
                    nc.scalar.dma_start(
                        out=outT[mt * 128:(mt + 1) * 128,
                                 boff * 512:boff * 512 + w],
                        in_=ost[:, 0:w])
            ctxB.close()

    nc.finalize()
    return nc


def _masks():
    p = np.arange(128)
    jj = np.arange(512)
    mq = np.zeros((128, 4, 512), np.float32)
    mk = np.zeros((128, 4, 512), np.float32)
    for r in range(4):
        rel = r * 128
        mq[:, r, :] = np.where((rel + p)[:, None] < jj[None, :], -50.0, 3.0e38)
        mk[:, r, :] = np.where((rel + p)[:, None] > jj[None, :], -300.0, 0.0)
    return mq, mk


_last_result = None


def kernel(x, wq, bq, wk, bk, wv, bv, wo, bo):
    global _last_result
    if "nc" not in _cache:
        _cache["nc"] = build()
        _cache["masks"] = _masks()
    nc = _cache["nc"]
    mq, mk = _cache["masks"]

    x = np.asarray(x, np.float32)
    in_maps = []
    for c in range(NCORE):
        b, g = divmod(c, 4)
        sl = slice(g * DG, (g + 1) * DG)
        in_maps.append({
            "xT": np.ascontiguousarray(x[b].T),
            "wq": np.ascontiguousarray(np.asarray(wq, np.float32)[:, sl]),
            "wk": np.ascontiguousarray(np.asarray(wk, np.float32)[:, sl]),
            "wv": np.ascontiguousarray(np.asarray(wv, np.float32)[:, sl]),
            "wo": np.ascontiguousarray(np.asarray(wo, np.float32)[sl, :]),
            "bq": np.ascontiguousarray(np.asarray(bq, np.float32)[sl, None]),
            "bk": np.ascontiguousarray(np.asarray(bk, np.float32)[sl, None]),
            "bv": np.ascontiguousarray(np.asarray(bv, np.float32)[sl, None]),
            "mq": mq, "mk": mk,
        })
    res = run_bass_kernel_spmd(nc, in_maps, core_ids=list(range(NCORE)))
    _last_result = res

    out = np.zeros((B, L, D), np.float32)
    attn = np.empty((B, H, L, L), np.float32)
    for c in range(NCORE):
        b, g = divmod(c, 4)
        r = res.results[c]
        attn[b, g * HPC:(g + 1) * HPC] = r["attn"]
        out[b] += r["outT"].T
    out += np.asarray(bo, np.float32)[None, None, :]
    return out, attn
